# revision 5
# baseline (speedup 1.0000x reference)
"""ConvCrossAttention Trainium2 kernel — self-contained.

Problem (B=4, C_in=C_out=256, H=W=64, N=4096):
  q = conv1x1(x1, Wq, bq); k = conv1x1(x2, Wk, bk); v = conv1x1(x2, Wv, bv)
  out = softmax(q^T k / sqrt(C)) @ v^T, back in conv layout [B, C, H, W].

Sharding: data-parallel over (batch, query-half) -> 8 NeuronCores.
Core c handles batch c//2, query rows (c%2)*2048 : (c%2+1)*2048, with the
full 4096-key context for that batch. No collectives.

Per-core program (everything SBUF-resident):
  Projections run in f32r (inputs arrive f32; PE f32r fast path).
  Q/K/V and P=exp(S/16) are stored bf16: SBUF traffic feeding the PE
  halves vs f32r, which is what lets the matmul stream run near the
  1 col/cycle PE rate (the f32r version measured ~40% slower streams).
  No max-subtraction needed before exp: |scores| < ~7.

  per 512-wide nq chunk, software-pipelined over 32 nk tiles:
    S^T[nk, nq] = K^T Q        (PE, 2 steps -> PSUM)
    P = exp(S^T / 16)          (ACT, bf16 out)
    acc += V^T[t] @ P          (PE, PSUM accumulate, trails by 2 tiles)
    psum_{p,d} += P            (Pool/DVE alternating, bf16)
  softmax tail per chunk (den = ones^T psum via PE; reciprocal_approx;
  broadcast via PE; normalize + bias on Pool/DVE) is emitted inside the
  NEXT chunk's tile loop so its serial latency never stalls the PE.
"""

import sys

if "/opt/trn_rl_repo" not in sys.path:
    sys.path.insert(0, "/opt/trn_rl_repo")

from contextlib import ExitStack

import numpy as np

import concourse.bass as bass  # noqa: F401
import concourse.mybir as mybir
import concourse.tile as tile
from concourse import bacc
from concourse.bass_utils import run_bass_kernel_spmd

F32 = mybir.dt.float32
F32R = mybir.dt.float32r
BF16 = mybir.dt.bfloat16

B, C, H, W = 4, 256, 64, 64
N = H * W  # 4096
NQ = 2048  # queries per core (half a batch)
NK = 4096  # full key context
CHUNK = 512
NQ_CHUNKS = NQ // CHUNK
NK_TILES = NK // 128
SCALE = 1.0 / 16.0  # C ** -0.5
PIPE = 2  # PV matmuls trail S matmuls by this many nk tiles
XDMA = 512  # input DMA chunk width


def build_nc():
    nc = bacc.Bacc(None, debug=False)

    x1 = nc.dram_tensor("x1c", [C, NQ], F32R, kind="ExternalInput")
    x2 = nc.dram_tensor("x2c", [C, NK], F32R, kind="ExternalInput")
    wq = nc.dram_tensor("wqT", [C, C], F32R, kind="ExternalInput")
    wk = nc.dram_tensor("wkT", [C, C], F32R, kind="ExternalInput")
    wv = nc.dram_tensor("wvT", [C, C], F32R, kind="ExternalInput")
    ball = nc.dram_tensor("ball", [C, 3], F32, kind="ExternalInput")
    out = nc.dram_tensor("out", [C, NQ], F32, kind="ExternalOutput")

    with tile.TileContext(nc) as tc, ExitStack() as ctx:
        big = ctx.enter_context(tc.tile_pool(name="big", bufs=1))
        small = ctx.enter_context(tc.tile_pool(name="small", bufs=1))
        ppool = ctx.enter_context(tc.tile_pool(name="p", bufs=4))
        opool = ctx.enter_context(tc.tile_pool(name="o", bufs=2))
        dpool = ctx.enter_context(tc.tile_pool(name="d", bufs=2))
        spsum = ctx.enter_context(tc.tile_pool(name="spsum", bufs=2, space="PSUM"))
        apsum = ctx.enter_context(tc.tile_pool(name="apsum", bufs=4, space="PSUM"))
        dpsum = ctx.enter_context(tc.tile_pool(name="dpsum", bufs=1, space="PSUM"))

        # --- weights / biases / constants (wk + first x2 chunk issued first
        # so the K projection can start as early as possible) ---
        wk_sb = small.tile([128, 2, C], F32R, tag="wk")
        wq_sb = small.tile([128, 2, C], F32R, tag="wq")
        wv_sb = small.tile([128, 2, C], F32R, tag="wv")
        b_sb = small.tile([128, 2, 3], F32, tag="b")
        x1_sb = big.tile([128, 2, NQ], F32R, tag="x1")
        x2_sb = big.tile([128, 2, NK], F32R, tag="x2")

        nc.sync.dma_start(out=wk_sb[:], in_=wk[:].rearrange("(h p) c -> p h c", p=128))
        nc.sync.dma_start(
            out=x2_sb[:, :, 0:XDMA],
            in_=x2[:, 0:XDMA].rearrange("(h p) n -> p h n", p=128),
        )
        nc.sync.dma_start(out=wv_sb[:], in_=wv[:].rearrange("(h p) c -> p h c", p=128))
        nc.sync.dma_start(out=wq_sb[:], in_=wq[:].rearrange("(h p) c -> p h c", p=128))
        nc.sync.dma_start(out=b_sb[:], in_=ball[:].rearrange("(h p) i -> p h i", p=128))

        ones_col_f32 = small.tile([128, 1], F32, tag="ones_col_f32")
        nc.vector.memset(ones_col_f32[:], 1.0)
        ones_col = small.tile([128, 1], BF16, tag="ones_col")
        nc.vector.tensor_copy(ones_col[:], ones_col_f32[:])
        ones_row_f32 = small.tile([1, 128], F32, tag="ones_row_f32")
        nc.vector.memset(ones_row_f32[:], 1.0)
        ones_row = small.tile([1, 128], BF16, tag="ones_row")
        nc.vector.tensor_copy(ones_row[:], ones_row_f32[:])

        # --- bf16 SBUF residents ---
        q_sb = big.tile([128, 2, NQ], BF16, tag="q")
        k_sb = big.tile([128, 2, NK], BF16, tag="k")
        v_sb = big.tile([128, NK_TILES, C], BF16, tag="v")

        # --- load x2 (+x1) chunks, project K and V^T as x2 arrives ---
        for j in range(NK // XDMA):
            if j + 1 < NK // XDMA:
                xs_n = slice((j + 1) * XDMA, (j + 2) * XDMA)
                nc.sync.dma_start(
                    out=x2_sb[:, :, xs_n],
                    in_=x2[:, xs_n].rearrange("(h p) n -> p h n", p=128),
                )
            if j < NQ // XDMA:
                xs1 = slice(j * XDMA, (j + 1) * XDMA)
                nc.sync.dma_start(
                    out=x1_sb[:, :, xs1],
                    in_=x1[:, xs1].rearrange("(h p) n -> p h n", p=128),
                )
            cs = slice(j * XDMA, (j + 1) * XDMA)
            for ct in range(2):
                kp = spsum.tile([128, XDMA], F32, tag="s", name="kp")
                ctslice = slice(ct * 128, (ct + 1) * 128)
                nc.tensor.matmul(
                    kp[:], wk_sb[:, 0, ctslice], x2_sb[:, 0, cs], start=True, stop=False
                )
                nc.tensor.matmul(
                    kp[:], wk_sb[:, 1, ctslice], x2_sb[:, 1, cs], start=False, stop=True
                )
                nc.vector.tensor_scalar_add(
                    k_sb[:, ct, cs], kp[:], b_sb[:, ct, 1:2]
                )
            for t in range(j * (XDMA // 128), (j + 1) * (XDMA // 128)):
                ts = slice(t * 128, (t + 1) * 128)
                vp = spsum.tile([128, C], F32, tag="s", name="vp")
                nc.tensor.matmul(
                    vp[:], x2_sb[:, 0, ts], wv_sb[:, 0, :], start=True, stop=False
                )
                nc.tensor.matmul(
                    vp[:], x2_sb[:, 1, ts], wv_sb[:, 1, :], start=False, stop=True
                )
                nc.scalar.copy(v_sb[:, t, :], vp[:])

        # --- project Q ---
        for j in range(NQ // XDMA):
            cs = slice(j * XDMA, (j + 1) * XDMA)
            for ct in range(2):
                qp = spsum.tile([128, XDMA], F32, tag="s", name="qp")
                ctslice = slice(ct * 128, (ct + 1) * 128)
                nc.tensor.matmul(
                    qp[:], wq_sb[:, 0, ctslice], x1_sb[:, 0, cs], start=True, stop=False
                )
                nc.tensor.matmul(
                    qp[:], wq_sb[:, 1, ctslice], x1_sb[:, 1, cs], start=False, stop=True
                )
                nc.vector.tensor_scalar_add(
                    q_sb[:, ct, cs], qp[:], b_sb[:, ct, 0:1]
                )

        # --- attention; each chunk's softmax tail is emitted inside the next
        # chunk's tile loop so the reciprocal chain never stalls the PE ---
        tail_den = tail_out = None
        for c0 in range(NQ_CHUNKS):
            cs = slice(c0 * CHUNK, (c0 + 1) * CHUNK)
            acc0 = apsum.tile([128, CHUNK], F32, tag="acc", name="acc0")
            acc1 = apsum.tile([128, CHUNK], F32, tag="acc", name="acc1")
            # P-sum split across Pool (even tiles) and DVE (odd tiles) so
            # neither engine's serial accumulation chain gates the PE.
            psum_p = dpool.tile([128, CHUNK], BF16, tag="psum_p", name="psum_p")
            psum_d = dpool.tile([128, CHUNK], BF16, tag="psum_d", name="psum_d")
            p_tiles = {}

            def emit_pv(t, acc0=acc0, acc1=acc1, psum_p=psum_p, psum_d=psum_d, p_tiles=p_tiles):
                first, last = t == 0, t == NK_TILES - 1
                p = p_tiles.pop(t)
                nc.tensor.matmul(
                    acc0[:], v_sb[:, t, 0:128], p[:], start=first, stop=last
                )
                nc.tensor.matmul(
                    acc1[:], v_sb[:, t, 128:256], p[:], start=first, stop=last
                )
                eng, acc_ps = (nc.gpsimd, psum_p) if t % 2 == 0 else (nc.vector, psum_d)
                if t < 2:
                    eng.tensor_copy(acc_ps[:], p[:])
                else:
                    eng.tensor_add(acc_ps[:], acc_ps[:], p[:])

            for t in range(NK_TILES):
                ts = slice(t * 128, (t + 1) * 128)
                sp = spsum.tile([128, CHUNK], F32, tag="s", name="sp")
                nc.tensor.matmul(
                    sp[:], k_sb[:, 0, ts], q_sb[:, 0, cs], start=True, stop=False
                )
                nc.tensor.matmul(
                    sp[:], k_sb[:, 1, ts], q_sb[:, 1, cs], start=False, stop=True
                )
                p = ppool.tile([128, CHUNK], BF16, tag="p", name="p")
                nc.scalar.activation(
                    p[:], sp[:], mybir.ActivationFunctionType.Exp, scale=SCALE
                )
                p_tiles[t] = p
                if t >= PIPE:
                    emit_pv(t - PIPE)
                if t == 1 and tail_den is not None:
                    tail_den()
                if t == 3 and tail_out is not None:
                    tail_out()

            for t in range(NK_TILES - PIPE, NK_TILES):
                emit_pv(t)

            def tail_den(psum_p=psum_p, psum_d=psum_d):
                den = dpsum.tile([1, CHUNK], F32, tag="den", name="den")
                nc.tensor.matmul(den[:], ones_col[:], psum_p[:], start=True, stop=False)
                nc.tensor.matmul(den[:], ones_col[:], psum_d[:], start=False, stop=True)
                recf = dpool.tile([1, CHUNK], F32, tag="recf", name="recf")
                nc.vector.reciprocal_approx_fast(out=recf[:], in_=den[:])
                rec_sb = dpool.tile([1, CHUNK], BF16, tag="rec_sb", name="rec_sb")
                nc.scalar.copy(rec_sb[:], recf[:])
                tail_den.rec_sb = rec_sb

            def tail_out(acc0=acc0, acc1=acc1, cs=cs, tail_den=tail_den):
                rec_sb = tail_den.rec_sb
                bc = dpsum.tile([128, CHUNK], F32, tag="bc", name="bc")
                nc.tensor.matmul(bc[:], ones_row[:], rec_sb[:], start=True, stop=True)
                bcs = opool.tile([128, CHUNK], F32, tag="bcs", name="bcs")
                nc.vector.tensor_copy(bcs[:], bc[:])
                t0 = opool.tile([128, CHUNK], F32, tag="t0", name="t0")
                nc.vector.tensor_mul(t0[:], acc0[:], bcs[:])
                t1 = opool.tile([128, CHUNK], F32, tag="t1", name="t1")
                nc.vector.tensor_mul(t1[:], acc1[:], bcs[:])
                o = opool.tile([128, 2, CHUNK], F32, tag="o", name="o")
                nc.vector.tensor_scalar_add(o[:, 0, :], t0[:], b_sb[:, 0, 2:3])
                nc.vector.tensor_scalar_add(o[:, 1, :], t1[:], b_sb[:, 1, 2:3])
                nc.sync.dma_start(
                    out=out[:, cs].rearrange("(h p) n -> p h n", p=128), in_=o[:]
                )

        # final chunk's tail
        tail_den()
        tail_out()

    nc.compile()
    return nc


def core_inputs(inputs, core):
    """Slice full-problem inputs for one core (numpy)."""
    b, h = core // 2, core % 2
    x1r = np.asarray(inputs["x1"], dtype=np.float32).reshape(B, C, N)
    x2r = np.asarray(inputs["x2"], dtype=np.float32).reshape(B, C, N)
    ball = np.stack(
        [
            np.asarray(inputs["bq"], dtype=np.float32),
            np.asarray(inputs["bk"], dtype=np.float32),
            np.asarray(inputs["bv"], dtype=np.float32),
        ],
        axis=1,
    )
    return {
        "x1c": np.ascontiguousarray(x1r[b][:, h * NQ : (h + 1) * NQ]),
        "x2c": np.ascontiguousarray(x2r[b]),
        "wqT": np.ascontiguousarray(np.asarray(inputs["Wq"], dtype=np.float32).T),
        "wkT": np.ascontiguousarray(np.asarray(inputs["Wk"], dtype=np.float32).T),
        "wvT": np.ascontiguousarray(np.asarray(inputs["Wv"], dtype=np.float32).T),
        "ball": np.ascontiguousarray(ball),
    }


_NC_CACHE = {}


def get_nc():
    if "nc" not in _NC_CACHE:
        _NC_CACHE["nc"] = build_nc()
    return _NC_CACHE["nc"]


def kernel(**inputs) -> np.ndarray:
    """Full-problem entry point: full inputs in, full [4,256,64,64] f32 out."""
    nc = get_nc()
    in_maps = [core_inputs(inputs, core) for core in range(8)]
    res = run_bass_kernel_spmd(nc, in_maps, list(range(8)))
    full = np.zeros((B, C, N), np.float32)
    for core in range(8):
        b, h = core // 2, core % 2
        full[b][:, h * NQ : (h + 1) * NQ] = res.results[core]["out"]
    return full.reshape(B, C, H, W)


# revision 8
# speedup vs baseline: 1.1107x; 1.1107x over previous
"""ConvCrossAttention Trainium2 kernel — self-contained.

Problem (B=4, C_in=C_out=256, H=W=64, N=4096):
  q = conv1x1(x1, Wq, bq); k = conv1x1(x2, Wk, bk); v = conv1x1(x2, Wv, bv)
  out = softmax(q^T k / sqrt(C)) @ v^T, back in conv layout [B, C, H, W].

Sharding: data-parallel over (batch, query-half) -> 8 NeuronCores.
Core c handles batch c//2, query rows (c%2)*2048 : (c%2+1)*2048, with the
full 4096-key context for that batch. No collectives.

Per-core program (everything SBUF-resident):
  Projections run in f32r (inputs arrive f32; PE f32r fast path).
  Q/K/V and P=exp(S/16) are stored bf16: SBUF traffic feeding the PE
  halves vs f32r, which is what lets the matmul stream run near the
  1 col/cycle PE rate (the f32r version measured ~40% slower streams).
  No max-subtraction needed before exp: |scores| < ~7.

  per 512-wide nq chunk, software-pipelined over 32 nk tiles:
    S^T[nk, nq] = K^T Q        (PE, 2 steps -> PSUM)
    P = exp(S^T / 16)          (ACT, bf16 out)
    acc += V^T[t] @ P          (PE, PSUM accumulate, trails by 2 tiles)
    psum_{p,d} += P            (Pool/DVE alternating, bf16)
  softmax tail per chunk (den = ones^T psum via PE; reciprocal_approx;
  broadcast via PE; normalize + bias on Pool/DVE) is emitted inside the
  NEXT chunk's tile loop so its serial latency never stalls the PE.
"""

import sys

if "/opt/trn_rl_repo" not in sys.path:
    sys.path.insert(0, "/opt/trn_rl_repo")

from contextlib import ExitStack

import numpy as np

import concourse.bass as bass  # noqa: F401
import concourse.mybir as mybir
import concourse.tile as tile
from concourse import bacc
from concourse.bass_utils import run_bass_kernel_spmd

F32 = mybir.dt.float32
F32R = mybir.dt.float32r
BF16 = mybir.dt.bfloat16

B, C, H, W = 4, 256, 64, 64
N = H * W  # 4096
NQ = 2048  # queries per core (half a batch)
NK = 4096  # full key context
CHUNK = 512
NQ_CHUNKS = NQ // CHUNK
NK_TILES = NK // 128
SCALE = 1.0 / 16.0  # C ** -0.5
PIPE = 2  # PV matmuls trail S matmuls by this many nk tiles
XDMA = 512  # input DMA chunk width


def build_nc():
    nc = bacc.Bacc(None, debug=False)

    x1 = nc.dram_tensor("x1c", [C, NQ], F32R, kind="ExternalInput")
    x2 = nc.dram_tensor("x2c", [C, NK], F32R, kind="ExternalInput")
    wq = nc.dram_tensor("wqT", [C, C], F32R, kind="ExternalInput")
    wk = nc.dram_tensor("wkT", [C, C], F32R, kind="ExternalInput")
    wv = nc.dram_tensor("wvT", [C, C], F32R, kind="ExternalInput")
    ball = nc.dram_tensor("ball", [C, 3], F32, kind="ExternalInput")
    out = nc.dram_tensor("out", [C, NQ], F32, kind="ExternalOutput")

    with tile.TileContext(nc) as tc, ExitStack() as ctx:
        big = ctx.enter_context(tc.tile_pool(name="big", bufs=1))
        small = ctx.enter_context(tc.tile_pool(name="small", bufs=1))
        ppool = ctx.enter_context(tc.tile_pool(name="p", bufs=4))
        opool = ctx.enter_context(tc.tile_pool(name="o", bufs=2))
        dpool = ctx.enter_context(tc.tile_pool(name="d", bufs=2))
        spsum = ctx.enter_context(tc.tile_pool(name="spsum", bufs=2, space="PSUM"))
        apsum = ctx.enter_context(tc.tile_pool(name="apsum", bufs=4, space="PSUM"))
        dpsum = ctx.enter_context(tc.tile_pool(name="dpsum", bufs=1, space="PSUM"))

        # --- weights / biases / constants (wk + first x2 chunk issued first
        # so the K projection can start as early as possible) ---
        wk_sb = small.tile([128, 2, C], F32R, tag="wk")
        wq_sb = small.tile([128, 2, C], F32R, tag="wq")
        wv_sb = small.tile([128, 2, C], F32R, tag="wv")
        b_sb = small.tile([128, 2, 3], F32, tag="b")
        x1_sb = big.tile([128, 2, NQ], F32R, tag="x1")
        x2_sb = big.tile([128, 2, NK], F32R, tag="x2")

        nc.sync.dma_start(out=wk_sb[:], in_=wk[:].rearrange("(h p) c -> p h c", p=128))
        nc.sync.dma_start(
            out=x2_sb[:, :, 0:XDMA],
            in_=x2[:, 0:XDMA].rearrange("(h p) n -> p h n", p=128),
        )
        nc.sync.dma_start(out=wv_sb[:], in_=wv[:].rearrange("(h p) c -> p h c", p=128))
        nc.sync.dma_start(out=wq_sb[:], in_=wq[:].rearrange("(h p) c -> p h c", p=128))
        nc.sync.dma_start(out=b_sb[:], in_=ball[:].rearrange("(h p) i -> p h i", p=128))

        ones_col_f32 = small.tile([128, 1], F32, tag="ones_col_f32")
        nc.vector.memset(ones_col_f32[:], 1.0)
        ones_col = small.tile([128, 1], BF16, tag="ones_col")
        nc.vector.tensor_copy(ones_col[:], ones_col_f32[:])
        ones_row_f32 = small.tile([1, 128], F32, tag="ones_row_f32")
        nc.vector.memset(ones_row_f32[:], 1.0)
        ones_row = small.tile([1, 128], F32R, tag="ones_row")
        nc.vector.tensor_copy(ones_row[:], ones_row_f32[:])

        # --- bf16 SBUF residents ---
        q_sb = big.tile([128, 2, NQ], BF16, tag="q")
        k_sb = big.tile([128, 2, NK], BF16, tag="k")
        v_sb = big.tile([128, NK_TILES, C], F32R, tag="v")

        # --- load x2 (+x1) chunks, project K and V^T as x2 arrives ---
        for j in range(NK // XDMA):
            if j + 1 < NK // XDMA:
                xs_n = slice((j + 1) * XDMA, (j + 2) * XDMA)
                nc.sync.dma_start(
                    out=x2_sb[:, :, xs_n],
                    in_=x2[:, xs_n].rearrange("(h p) n -> p h n", p=128),
                )
            if j < NQ // XDMA:
                xs1 = slice(j * XDMA, (j + 1) * XDMA)
                nc.sync.dma_start(
                    out=x1_sb[:, :, xs1],
                    in_=x1[:, xs1].rearrange("(h p) n -> p h n", p=128),
                )
            cs = slice(j * XDMA, (j + 1) * XDMA)
            for ct in range(2):
                kp = spsum.tile([128, XDMA], F32, tag="s", name="kp")
                ctslice = slice(ct * 128, (ct + 1) * 128)
                nc.tensor.matmul(
                    kp[:], wk_sb[:, 0, ctslice], x2_sb[:, 0, cs], start=True, stop=False
                )
                nc.tensor.matmul(
                    kp[:], wk_sb[:, 1, ctslice], x2_sb[:, 1, cs], start=False, stop=True
                )
                nc.vector.tensor_scalar_add(
                    k_sb[:, ct, cs], kp[:], b_sb[:, ct, 1:2]
                )
            for t in range(j * (XDMA // 128), (j + 1) * (XDMA // 128)):
                ts = slice(t * 128, (t + 1) * 128)
                vp = spsum.tile([128, C], F32, tag="s", name="vp")
                nc.tensor.matmul(
                    vp[:], x2_sb[:, 0, ts], wv_sb[:, 0, :], start=True, stop=False
                )
                nc.tensor.matmul(
                    vp[:], x2_sb[:, 1, ts], wv_sb[:, 1, :], start=False, stop=True
                )
                nc.scalar.copy(v_sb[:, t, :], vp[:])

        # --- project Q ---
        for j in range(NQ // XDMA):
            cs = slice(j * XDMA, (j + 1) * XDMA)
            for ct in range(2):
                qp = spsum.tile([128, XDMA], F32, tag="s", name="qp")
                ctslice = slice(ct * 128, (ct + 1) * 128)
                nc.tensor.matmul(
                    qp[:], wq_sb[:, 0, ctslice], x1_sb[:, 0, cs], start=True, stop=False
                )
                nc.tensor.matmul(
                    qp[:], wq_sb[:, 1, ctslice], x1_sb[:, 1, cs], start=False, stop=True
                )
                nc.vector.tensor_scalar_add(
                    q_sb[:, ct, cs], qp[:], b_sb[:, ct, 0:1]
                )

        # --- attention; each chunk's softmax tail is emitted inside the next
        # chunk's tile loop so the reciprocal chain never stalls the PE ---
        tail_den = tail_out = None
        for c0 in range(NQ_CHUNKS):
            cs = slice(c0 * CHUNK, (c0 + 1) * CHUNK)
            acc0 = apsum.tile([128, CHUNK], F32, tag="acc", name="acc0")
            acc1 = apsum.tile([128, CHUNK], F32, tag="acc", name="acc1")
            # P-sum split across Pool (even tiles) and DVE (odd tiles) so
            # neither engine's serial accumulation chain gates the PE.
            psum_p = dpool.tile([128, CHUNK], BF16, tag="psum_p", name="psum_p")
            psum_d = dpool.tile([128, CHUNK], BF16, tag="psum_d", name="psum_d")
            p_tiles = {}

            def emit_pv(t, acc0=acc0, acc1=acc1, psum_p=psum_p, psum_d=psum_d, p_tiles=p_tiles):
                first, last = t == 0, t == NK_TILES - 1
                p = p_tiles.pop(t)
                nc.tensor.matmul(
                    acc0[:], v_sb[:, t, 0:128], p[:], start=first, stop=last
                )
                nc.tensor.matmul(
                    acc1[:], v_sb[:, t, 128:256], p[:], start=first, stop=last
                )
                eng, acc_ps = (nc.gpsimd, psum_p) if t % 2 == 0 else (nc.vector, psum_d)
                if t < 2:
                    eng.tensor_copy(acc_ps[:], p[:].bitcast(F32))
                else:
                    eng.tensor_add(acc_ps[:], acc_ps[:], p[:].bitcast(F32))

            for t in range(NK_TILES):
                ts = slice(t * 128, (t + 1) * 128)
                sp = spsum.tile([128, CHUNK], F32, tag="s", name="sp")
                nc.tensor.matmul(
                    sp[:], k_sb[:, 0, ts], q_sb[:, 0, cs], start=True, stop=False
                )
                nc.tensor.matmul(
                    sp[:], k_sb[:, 1, ts], q_sb[:, 1, cs], start=False, stop=True
                )
                p = ppool.tile([128, CHUNK], F32R, tag="p", name="p")
                nc.scalar.activation(
                    p[:], sp[:], mybir.ActivationFunctionType.Exp, scale=SCALE
                )
                p_tiles[t] = p
                if t >= PIPE:
                    emit_pv(t - PIPE)
                if t == 1 and tail_den is not None:
                    tail_den()
                if t == 3 and tail_out is not None:
                    tail_out()

            for t in range(NK_TILES - PIPE, NK_TILES):
                emit_pv(t)

            def tail_den(psum_p=psum_p, psum_d=psum_d):
                den = dpsum.tile([1, CHUNK], F32, tag="den", name="den")
                nc.tensor.matmul(den[:], ones_col[:], psum_p[:], start=True, stop=False)
                nc.tensor.matmul(den[:], ones_col[:], psum_d[:], start=False, stop=True)
                recf = dpool.tile([1, CHUNK], F32, tag="recf", name="recf")
                nc.vector.reciprocal_approx_fast(out=recf[:], in_=den[:])
                rec_r = dpool.tile([1, CHUNK], F32R, tag="rec_r", name="rec_r")
                nc.vector.tensor_copy(rec_r[:], recf[:])
                tail_den.recf = rec_r

            def tail_out(acc0=acc0, acc1=acc1, cs=cs, tail_den=tail_den):
                recf = tail_den.recf
                bc = dpsum.tile([128, CHUNK], F32, tag="bc", name="bc")
                nc.tensor.matmul(bc[:], ones_row[:], recf[:], start=True, stop=True)
                bcs = opool.tile([128, CHUNK], F32, tag="bcs", name="bcs")
                nc.vector.tensor_copy(bcs[:], bc[:])
                t0 = opool.tile([128, CHUNK], F32, tag="t0", name="t0")
                nc.vector.tensor_mul(t0[:], acc0[:], bcs[:])
                t1 = opool.tile([128, CHUNK], F32, tag="t1", name="t1")
                nc.vector.tensor_mul(t1[:], acc1[:], bcs[:])
                o = opool.tile([128, 2, CHUNK], F32, tag="o", name="o")
                nc.vector.tensor_scalar_add(o[:, 0, :], t0[:], b_sb[:, 0, 2:3])
                nc.vector.tensor_scalar_add(o[:, 1, :], t1[:], b_sb[:, 1, 2:3])
                nc.sync.dma_start(
                    out=out[:, cs].rearrange("(h p) n -> p h n", p=128), in_=o[:]
                )

        # final chunk's tail
        tail_den()
        tail_out()

    nc.compile()
    return nc


def core_inputs(inputs, core):
    """Slice full-problem inputs for one core (numpy)."""
    b, h = core // 2, core % 2
    x1r = np.asarray(inputs["x1"], dtype=np.float32).reshape(B, C, N)
    x2r = np.asarray(inputs["x2"], dtype=np.float32).reshape(B, C, N)
    ball = np.stack(
        [
            np.asarray(inputs["bq"], dtype=np.float32),
            np.asarray(inputs["bk"], dtype=np.float32),
            np.asarray(inputs["bv"], dtype=np.float32),
        ],
        axis=1,
    )
    return {
        "x1c": np.ascontiguousarray(x1r[b][:, h * NQ : (h + 1) * NQ]),
        "x2c": np.ascontiguousarray(x2r[b]),
        "wqT": np.ascontiguousarray(np.asarray(inputs["Wq"], dtype=np.float32).T),
        "wkT": np.ascontiguousarray(np.asarray(inputs["Wk"], dtype=np.float32).T),
        "wvT": np.ascontiguousarray(np.asarray(inputs["Wv"], dtype=np.float32).T),
        "ball": np.ascontiguousarray(ball),
    }


_NC_CACHE = {}


def get_nc():
    if "nc" not in _NC_CACHE:
        _NC_CACHE["nc"] = build_nc()
    return _NC_CACHE["nc"]


def kernel(**inputs) -> np.ndarray:
    """Full-problem entry point: full inputs in, full [4,256,64,64] f32 out."""
    nc = get_nc()
    in_maps = [core_inputs(inputs, core) for core in range(8)]
    res = run_bass_kernel_spmd(nc, in_maps, list(range(8)))
    full = np.zeros((B, C, N), np.float32)
    for core in range(8):
        b, h = core // 2, core % 2
        full[b][:, h * NQ : (h + 1) * NQ] = res.results[core]["out"]
    return full.reshape(B, C, H, W)
